# revision 5
# baseline (speedup 1.0000x reference)
"""Multi-head attention (batch=2, seq=2048, dim=256, nhead=8, head_dim=256)
distributed across 8 trn2 NeuronCores.

Sharding: the 16 (batch, head) pairs are distributed 2-per-core (cores 0-3
handle batch 0 heads 0-7, cores 4-7 batch 1). Each core computes its two
heads' projections + attention + output-projection partial; the host sums
the 4 partials per batch and adds the output bias.

On-device layout (per core, all matmul operands bf16, PSUM accum fp32):
  xt  [2,128,2048]  x[b].T        (e on partitions, split into 2 tiles)
  wq/wk/wv [2,2,128,256]  W_h.T   (e on partitions, d free)
  wo  [2,2,128,256]  Wo[:,hslice].T (d on partitions, o free)
  qT/kT [d=256, s=2048] = (Wh.T-tile).T @ xT      -> scoresT = kT.T-tile @ qT
  expT via ScalarE Exp(scale=1/16) straight out of PSUM (scores |s|<~1, no
  max-subtraction needed), denom = ones.T @ (bf16 add-tree of expT),
  outU.T = v-tile.T @ expT, normalized with a broadcast-matmul of 1/denom,
  final = outU.T-tile.T @ wo accumulated over the 2 local heads.
"""

import sys

if "/opt/trn_rl_repo" not in sys.path:
    sys.path.insert(0, "/opt/trn_rl_repo")

import numpy as np
import ml_dtypes

P = 128
S = 2048
D = 256
CHUNK = 512
CH = S // CHUNK  # 4 sq chunks
NKT = S // P     # 16 sk tiles
NHEAD = 8
NCORES = 8

_BUILT = None


def _build():
    import concourse.bacc as bacc
    import concourse.mybir as mybir
    import concourse.tile as tile
    from contextlib import ExitStack

    BF = mybir.dt.bfloat16
    F32 = mybir.dt.float32
    EXP = mybir.ActivationFunctionType.Exp

    nc = bacc.Bacc(None, target_bir_lowering=False, debug=False)
    with tile.TileContext(nc) as tc:
        with ExitStack() as ctx:
            dram = ctx.enter_context(tc.tile_pool(name="dram", bufs=1, space="DRAM"))
            xt_d = dram.tile([2, P, S], BF, kind="ExternalInput", name="xt")
            wq_d = dram.tile([2, 2, P, D], BF, kind="ExternalInput", name="wq")
            wk_d = dram.tile([2, 2, P, D], BF, kind="ExternalInput", name="wk")
            wv_d = dram.tile([2, 2, P, D], BF, kind="ExternalInput", name="wv")
            wo_d = dram.tile([2, 2, P, D], BF, kind="ExternalInput", name="wo")
            out_d = dram.tile([S, D], F32, kind="ExternalOutput", name="out")

            const = ctx.enter_context(tc.tile_pool(name="const", bufs=1))
            ones_bf = const.tile([P, 1], BF, name="ones_bf")
            nc.vector.memset(ones_bf[:], 1.0)

            xpool = ctx.enter_context(tc.tile_pool(name="xtp", bufs=1))
            xt_sb = []
            for et in range(2):
                t = xpool.tile([P, S], BF, name=f"xt{et}")
                nc.sync.dma_start(out=t[:], in_=xt_d[et])
                xt_sb.append(t)

            wpool = ctx.enter_context(tc.tile_pool(name="wp", bufs=1))
            w_sb = {}
            for nm, src in (("wq", wq_d), ("wk", wk_d), ("wv", wv_d), ("wo", wo_d)):
                for j in range(2):
                    for et in range(2):
                        t = wpool.tile([P, D], BF, name=f"{nm}{j}{et}")
                        nc.sync.dma_start(out=t[:], in_=src[j, et])
                        w_sb[(nm, j, et)] = t

            fpool = ctx.enter_context(tc.tile_pool(name="fp", bufs=1))
            final_sb = fpool.tile([P, NKT * D], F32, name="final")

            qkpool = ctx.enter_context(tc.tile_pool(name="qkp", bufs=2))
            vpool = ctx.enter_context(tc.tile_pool(name="vp", bufs=2))
            epool = ctx.enter_context(tc.tile_pool(name="ep", bufs=2))
            tpool = ctx.enter_context(tc.tile_pool(name="tp", bufs=2))
            rpool = ctx.enter_context(tc.tile_pool(name="rp", bufs=2))
            opool = ctx.enter_context(tc.tile_pool(name="op", bufs=2))

            psA = ctx.enter_context(tc.tile_pool(name="psA", bufs=2, space="PSUM"))
            psB = ctx.enter_context(tc.tile_pool(name="psB", bufs=3, space="PSUM"))
            psD = ctx.enter_context(tc.tile_pool(name="psD", bufs=1, space="PSUM"))

            for j in range(2):
                # ---- q/k projections: qT/kT [d=256, s=2048] (bf16) ----
                qt_sb = [qkpool.tile([P, S], BF, tag=f"qt{dt}", name=f"qt{dt}_{j}") for dt in range(2)]
                kt_sb = [qkpool.tile([P, S], BF, tag=f"kt{dt}", name=f"kt{dt}_{j}") for dt in range(2)]
                for dst, wname in ((qt_sb, "wq"), (kt_sb, "wk")):
                    for dt in range(2):
                        for c in range(CH):
                            ps = psB.tile([P, CHUNK], F32, tag="psB", name="ps_proj")
                            for et in range(2):
                                nc.tensor.matmul(
                                    ps[:],
                                    lhsT=w_sb[(wname, j, et)][:, dt * P:(dt + 1) * P],
                                    rhs=xt_sb[et][:, c * CHUNK:(c + 1) * CHUNK],
                                    start=(et == 0), stop=(et == 1),
                                )
                            nc.vector.tensor_copy(dst[dt][:, c * CHUNK:(c + 1) * CHUNK], ps[:])
                # ---- v projection: v [s, d] natural layout (bf16) ----
                v_sb = vpool.tile([P, NKT * D], BF, tag="v", name=f"v_{j}")
                for st in range(NKT):
                    ps = psB.tile([P, CHUNK], F32, tag="psB", name="ps_v")
                    for et in range(2):
                        nc.tensor.matmul(
                            ps[:, :D],
                            lhsT=xt_sb[et][:, st * P:(st + 1) * P],
                            rhs=w_sb[("wv", j, et)][:],
                            start=(et == 0), stop=(et == 1),
                        )
                    nc.scalar.copy(v_sb[:, st * D:(st + 1) * D], ps[:, :D])

                # ---- attention, chunked over sq ----
                outu_sb = [opool.tile([P, S], BF, tag=f"ou{dt}", name=f"ou{dt}_{j}") for dt in range(2)]
                for c in range(CH):
                    E = epool.tile([P, NKT * CHUNK], BF, tag="E", name=f"E_{j}_{c}")
                    for g in range(NKT // 2):
                        ps = psA.tile([P, 2 * CHUNK], F32, tag="psA", name="ps_qk")
                        for half in range(2):
                            kt_idx = 2 * g + half
                            for dt in range(2):
                                nc.tensor.matmul(
                                    ps[:, half * CHUNK:(half + 1) * CHUNK],
                                    lhsT=kt_sb[dt][:, kt_idx * P:(kt_idx + 1) * P],
                                    rhs=qt_sb[dt][:, c * CHUNK:(c + 1) * CHUNK],
                                    start=(dt == 0), stop=(dt == 1),
                                )
                        nc.scalar.activation(
                            E[:, g * 2 * CHUNK:(g + 1) * 2 * CHUNK], ps[:],
                            EXP, scale=1.0 / 16.0,
                        )
                    # denominator: bf16 add-tree then ones-matmul partition sum
                    t1 = tpool.tile([P, 8 * CHUNK], BF, tag="t1", name="t1")
                    nc.vector.tensor_add(t1[:], E[:, :8 * CHUNK], E[:, 8 * CHUNK:])
                    t2 = tpool.tile([P, 4 * CHUNK], BF, tag="t2", name="t2")
                    nc.vector.tensor_add(t2[:], t1[:, :4 * CHUNK], t1[:, 4 * CHUNK:])
                    t3 = tpool.tile([P, 2 * CHUNK], BF, tag="t3", name="t3")
                    nc.vector.tensor_add(t3[:], t2[:, :2 * CHUNK], t2[:, 2 * CHUNK:])
                    t4 = tpool.tile([P, CHUNK], BF, tag="t4", name="t4")
                    nc.vector.tensor_add(t4[:], t3[:, :CHUNK], t3[:, CHUNK:])
                    psd = psD.tile([1, CHUNK], F32, tag="psD", name="ps_d")
                    nc.tensor.matmul(psd[:], lhsT=ones_bf[:], rhs=t4[:], start=True, stop=True)
                    recip = rpool.tile([1, CHUNK], F32, tag="recip", name="recip")
                    nc.vector.reciprocal(recip[:], psd[:])
                    rb = rpool.tile([P, CHUNK], F32, tag="rb", name="rb")
                    nc.gpsimd.partition_broadcast(rb[:], recip[:])
                    # AV: outU.T [d, sq] accumulated over sk tiles, then normalize
                    for dt in range(2):
                        ps = psB.tile([P, CHUNK], F32, tag="psB", name="ps_av")
                        for kt_idx in range(NKT):
                            nc.tensor.matmul(
                                ps[:],
                                lhsT=v_sb[:, kt_idx * D + dt * P: kt_idx * D + dt * P + P],
                                rhs=E[:, kt_idx * CHUNK:(kt_idx + 1) * CHUNK],
                                start=(kt_idx == 0), stop=(kt_idx == NKT - 1),
                            )
                        nc.vector.tensor_mul(outu_sb[dt][:, c * CHUNK:(c + 1) * CHUNK], ps[:], rb[:])

                # ---- output projection, accumulated over the 2 local heads ----
                for st in range(NKT):
                    ps = psB.tile([P, CHUNK], F32, tag="psB", name="ps_o")
                    for dt in range(2):
                        nc.tensor.matmul(
                            ps[:, :D],
                            lhsT=outu_sb[dt][:, st * P:(st + 1) * P],
                            rhs=w_sb[("wo", j, dt)][:],
                            start=(dt == 0), stop=(dt == 1),
                        )
                    if j == 0:
                        nc.scalar.copy(final_sb[:, st * D:(st + 1) * D], ps[:, :D])
                    else:
                        nc.vector.tensor_add(
                            final_sb[:, st * D:(st + 1) * D],
                            final_sb[:, st * D:(st + 1) * D],
                            ps[:, :D],
                        )
                        nc.sync.dma_start(
                            out=out_d[st * P:(st + 1) * P, :],
                            in_=final_sb[:, st * D:(st + 1) * D],
                        )
    nc.compile()
    names = dict(xt=xt_d.name, wq=wq_d.name, wk=wk_d.name, wv=wv_d.name,
                 wo=wo_d.name, out=out_d.name)
    return nc, names


def _get_built():
    global _BUILT
    if _BUILT is None:
        _BUILT = _build()
    return _BUILT


def _prep_core_inputs(i, x, Wq, Wk, Wv, Wo, names):
    bf16 = ml_dtypes.bfloat16
    b = i // 4
    heads = [(2 * i) % NHEAD, (2 * i) % NHEAD + 1]
    xt = np.ascontiguousarray(x[b].T).reshape(2, P, S).astype(bf16)

    def head_T(W, h):  # W[h*D:(h+1)*D, :].T -> [e=256, d=256] -> [2,128,256]
        return np.ascontiguousarray(W[h * D:(h + 1) * D, :].T).reshape(2, P, D)

    wq = np.stack([head_T(Wq, h) for h in heads]).astype(bf16)
    wk = np.stack([head_T(Wk, h) for h in heads]).astype(bf16)
    wv = np.stack([head_T(Wv, h) for h in heads]).astype(bf16)
    wo = np.stack(
        [np.ascontiguousarray(Wo[:, h * D:(h + 1) * D].T).reshape(2, P, D) for h in heads]
    ).astype(bf16)
    return {names["xt"]: xt, names["wq"]: wq, names["wk"]: wk,
            names["wv"]: wv, names["wo"]: wo}


def kernel(x, Wq, Wk, Wv, Wo, bo):
    from concourse.bass_utils import run_bass_kernel_spmd

    x = np.asarray(x, dtype=np.float32)
    Wq = np.asarray(Wq, dtype=np.float32)
    Wk = np.asarray(Wk, dtype=np.float32)
    Wv = np.asarray(Wv, dtype=np.float32)
    Wo = np.asarray(Wo, dtype=np.float32)
    bo = np.asarray(bo, dtype=np.float32)

    nc, names = _get_built()
    in_maps = [_prep_core_inputs(i, x, Wq, Wk, Wv, Wo, names) for i in range(NCORES)]
    res = run_bass_kernel_spmd(nc, in_maps, core_ids=list(range(NCORES)))

    out = np.zeros((2, S, D), dtype=np.float32)
    for b in range(2):
        acc = np.zeros((S, D), dtype=np.float32)
        for i in range(4 * b, 4 * b + 4):
            acc += res.results[i][names["out"]]
        out[b] = acc + bo[None, :]
    return out
